# revision 18
# baseline (speedup 1.0000x reference)
"""Trainium2 Bass kernel for nn_NetAtom (Behler-Parrinello segment reduce).

Full-input contract: kernel(**inputs) takes the complete (unsharded) numpy
arrays from setup_inputs() and returns the full [2K] output.

Strategy (8 cores, atom sharding):
  - Host: transpose desc ([N,D] -> [D,N]) and logic ([K,N] -> [N,K]), cast to
    bf16, pad atoms to a multiple of 128 per core (padded logic rows are zero
    so padded atoms contribute nothing), shard atoms across the 8 cores.
  - Device (per core, all-bf16 matmuls with fp32 PSUM accumulation):
      h1T = tanh(W1 @ descT + b1)          [256, n]  (2 partition tiles)
      h2T = tanh(W2 @ h1T + b2)            [256, n]
      per 128-atom subchunk j:  pv[n,2] = h2T_j.T @ W3T
      v[:,0] = pv[:,0] + b3[0]   (DVE)
      v[:,1] = softplus(pv[:,1] + b3[1]) = Ln(Exp(.) + 1)  (ACT)
      psum[2,500] (x2 banks) += v_j.T @ logicT_j   accumulated over all
      subchunks of both species.
  - The Ln lives in a different ACT function set than Tanh/Exp, and each set
    switch costs a ~1.3us table load.  Chunks are therefore processed in
    groups of G=16: Tanh/Exp run per chunk, the Ln runs once per group, and
    the matvec of the group's chunks follows (logic tiles stay resident, so
    the logic pool is G+2 deep).
  - Host: sum the 8 per-core [2,1000] partials, concat -> [2000].
"""

import numpy as np
import ml_dtypes

import concourse.mybir as mybir
import concourse.tile as tile
from concourse import bacc
from concourse.bass_utils import run_bass_kernel_spmd

BF = mybir.dt.bfloat16
F8 = mybir.dt.float8e4
F32 = mybir.dt.float32
ACTF = mybir.ActivationFunctionType

D = 128        # descriptor size
H = 256        # hidden width
N = 100000     # atoms per species (full)
K = 1000       # structures
NCORES = 8
NA = 12544     # atoms per core, padded (= 98 * 128); 8*12544 = 100352 >= N
CHUNK = 512    # atoms per pipeline chunk
NJMAX = CHUNK // 128
KH = K // 2    # structure half (one PSUM bank each)
G = 8          # chunks per Ln group
MV_DRAIN = 2   # matvec chunks emitted per pipeline slot

# flat chunk list over both species
_CHUNKS = []
for _s in (0, 1):
    _off = 0
    while _off < NA:
        _cs = min(CHUNK, NA - _off)
        _CHUNKS.append((_s, _off, _cs))
        _off += _cs
_NJ_TOTAL = sum(cs // 128 for _, _, cs in _CHUNKS)


def build_nc(repeat=None):
    nc = bacc.Bacc()

    ins = {}
    for s in (0, 1):
        ins[f"descT{s}"] = nc.dram_tensor(f"descT{s}", [D, NA], BF,
                                          kind="ExternalInput")
        ins[f"logicT{s}"] = nc.dram_tensor(f"logicT{s}", [NA, K], F8,
                                           kind="ExternalInput")
        ins[f"w1t{s}"] = nc.dram_tensor(f"w1t{s}", [D, H], BF,
                                        kind="ExternalInput")
        ins[f"w2t{s}"] = nc.dram_tensor(f"w2t{s}", [128, 2, H], BF,
                                        kind="ExternalInput")
        ins[f"w3t{s}"] = nc.dram_tensor(f"w3t{s}", [128, 2, 2], BF,
                                        kind="ExternalInput")
        ins[f"b1{s}"] = nc.dram_tensor(f"b1{s}", [128, 2], F32,
                                       kind="ExternalInput")
        ins[f"b2{s}"] = nc.dram_tensor(f"b2{s}", [128, 2], F32,
                                       kind="ExternalInput")
        ins[f"b3{s}"] = nc.dram_tensor(f"b3{s}", [128, 2], F32,
                                       kind="ExternalInput")
    out_d = nc.dram_tensor("out", [2, K], F32, kind="ExternalOutput")

    with tile.TileContext(nc) as tc:
        with tc.tile_pool(name="consts", bufs=1) as consts, \
             tc.tile_pool(name="descp", bufs=4) as descp, \
             tc.tile_pool(name="logicp", bufs=2 * G) as logicp, \
             tc.tile_pool(name="hp", bufs=4) as hp, \
             tc.tile_pool(name="vp", bufs=3) as vp, \
             tc.tile_pool(name="outp", bufs=1) as outp, \
             tc.tile_pool(name="ps_mlp", bufs=5, space="PSUM") as ps_mlp, \
             tc.tile_pool(name="ps_v", bufs=1, space="PSUM") as ps_v, \
             tc.tile_pool(name="ps_mv", bufs=1, space="PSUM") as ps_mv:

            import contextlib
            _stack = contextlib.ExitStack()
            if repeat:
                _stack.enter_context(tc.For_i(0, repeat, 1))

            # ---- constants, loaded once ----
            w1, w2, w3, b1, b2, b3 = {}, {}, {}, {}, {}, {}
            for s in (0, 1):
                w1[s] = consts.tile([D, H], BF, name=f"w1_{s}")
                nc.sync.dma_start(out=w1[s], in_=ins[f"w1t{s}"][:, :])
                w2[s] = consts.tile([128, 2, H], BF, name=f"w2_{s}")
                nc.sync.dma_start(out=w2[s], in_=ins[f"w2t{s}"][:, :, :])
                w3[s] = consts.tile([128, 2, 2], BF, name=f"w3_{s}")
                nc.sync.dma_start(out=w3[s], in_=ins[f"w3t{s}"][:, :, :])
                b1[s] = consts.tile([128, 2], F32, name=f"b1_{s}")
                nc.sync.dma_start(out=b1[s], in_=ins[f"b1{s}"][:, :])
                b2[s] = consts.tile([128, 2], F32, name=f"b2_{s}")
                nc.sync.dma_start(out=b2[s], in_=ins[f"b2{s}"][:, :])
                b3[s] = consts.tile([128, 2], F32, name=f"b3_{s}")
                nc.sync.dma_start(out=b3[s], in_=ins[f"b3{s}"][:, :])

            # ---- matvec accumulators: [2, 500] x2, live for whole kernel ----
            pmv = [ps_mv.tile([2, KH], F32, name=f"pmv{h}") for h in (0, 1)]

            n_chunks = len(_CHUNKS)
            mv_emitted = [0]
            last_mv = [None]

            def stage_a(cdesc):
                """DMA loads + layer 1 + tanh(h1)."""
                s, n0, cs = cdesc
                nj = cs // 128
                lt = logicp.tile([128, NJMAX, K], F8, name="lt", tag="lt")
                nc.sync.dma_start(
                    out=lt[:, :nj, :],
                    in_=ins[f"logicT{s}"][n0:n0 + cs, :]
                        .rearrange("(j p) k -> p j k", p=128),
                )
                dt = descp.tile([D, CHUNK], BF, name="dt", tag="dt")
                nc.sync.dma_start(out=dt[:, :cs],
                                  in_=ins[f"descT{s}"][:, n0:n0 + cs])
                h1 = hp.tile([128, 2, CHUNK], BF, name="h1", tag="h1")
                for ht in (0, 1):
                    p1 = ps_mlp.tile([128, CHUNK], F32, name="pmlp",
                                     tag="pmlp")
                    nc.tensor.matmul(
                        p1[:, :cs],
                        lhsT=w1[s][:, ht * 128:(ht + 1) * 128],
                        rhs=dt[:, :cs],
                        start=True, stop=True,
                    )
                    nc.scalar.activation(
                        h1[:, ht, :cs], p1[:, :cs], ACTF.Tanh,
                        bias=b1[s][:, ht:ht + 1], scale=1.0,
                    )
                return dict(s=s, cs=cs, nj=nj, lt=lt, h1=h1)

            def stage_b(meta):
                """Layer 2 + tanh(h2)."""
                s, cs, h1 = meta["s"], meta["cs"], meta["h1"]
                h2 = hp.tile([128, 2, CHUNK], BF, name="h2", tag="h2")
                for ht in (0, 1):
                    p2 = ps_mlp.tile([128, CHUNK], F32, name="pmlp",
                                     tag="pmlp")
                    for kk in (0, 1):
                        nc.tensor.matmul(
                            p2[:, :cs],
                            lhsT=w2[s][:, kk, ht * 128:(ht + 1) * 128],
                            rhs=h1[:, kk, :cs],
                            start=(kk == 0), stop=(kk == 1),
                        )
                    nc.scalar.activation(
                        h2[:, ht, :cs], p2[:, :cs], ACTF.Tanh,
                        bias=b2[s][:, ht:ht + 1], scale=1.0,
                    )
                meta["h2"] = h2

            def stage_c(meta, grp):
                """Layer 3 + v-even (DVE) + exp stash."""
                s, nj, h2 = meta["s"], meta["nj"], meta["h2"]
                pv = ps_v.tile([128, 2 * NJMAX], F32, name="pv", tag="pv")
                for j in range(nj):
                    for kk in (0, 1):
                        mm = nc.tensor.matmul(
                            pv[:, 2 * j:2 * j + 2],
                            lhsT=h2[:, kk, j * 128:(j + 1) * 128],
                            rhs=w3[s][:, kk, :],
                            start=(kk == 0), stop=(kk == 1),
                        )
                        # keep L3 behind this slot's matvec burst in the PE
                        # stream: its tanh(h2) input lands late, and hoisting
                        # it ahead of ready matvec work stalls the PE.
                        if j == 0 and kk == 0 and last_mv[0] is not None:
                            tile.add_dep_helper(
                                mm.ins, last_mv[0].ins, sync=False,
                                reason="order L3 after matvec burst")

                jj = grp["jj"]
                nc.vector.tensor_scalar_add(
                    grp["vg"][:, 2 * jj:2 * (jj + nj):2],
                    pv[:, 0:2 * nj:2],
                    b3[s][:, 0:1],
                )
                nc.scalar.activation(
                    grp["tg"][:, jj:jj + nj], pv[:, 1:2 * nj:2], ACTF.Exp,
                    bias=b3[s][:, 1:2], scale=1.0,
                )
                meta["vg"] = grp["vg"]
                meta["jj"] = jj
                grp["jj"] = jj + nj

            def emit_ln(grp):
                gnj = grp["jj"]
                nc.scalar.activation(
                    grp["vg"][:, 1:2 * gnj:2], grp["tg"][:, :gnj], ACTF.Ln,
                    bias=1.0, scale=1.0,
                )

            def emit_mv(meta):
                nj, lt, vg, jj = meta["nj"], meta["lt"], meta["vg"], meta["jj"]
                first = mv_emitted[0] == 0
                last = mv_emitted[0] == n_chunks - 1
                for j in range(nj):
                    for h in (0, 1):
                        last_mv[0] = nc.tensor.matmul(
                            pmv[h][:, :],
                            lhsT=vg[:, 2 * (jj + j):2 * (jj + j) + 2],
                            rhs=lt[:, j, h * KH:(h + 1) * KH],
                            start=(first and j == 0),
                            stop=(last and j == nj - 1),
                            skip_group_check=True,
                        )
                mv_emitted[0] += 1

            def new_grp():
                return dict(
                    vg=vp.tile([128, 2 * G * NJMAX], F8, name="vg", tag="vg"),
                    tg=vp.tile([128, G * NJMAX], F32, name="tg", tag="tg"),
                    jj=0, metas=[],
                )

            from collections import deque
            pending = deque()
            prev_a = None
            prev_b = None
            grp = None
            for ci in range(n_chunks + 2):
                meta = stage_a(_CHUNKS[ci]) if ci < n_chunks else None
                if prev_a is not None:
                    stage_b(prev_a)
                for _ in range(MV_DRAIN):
                    if pending:
                        emit_mv(pending.popleft())
                if prev_b is not None:
                    if grp is None:
                        grp = new_grp()
                    stage_c(prev_b, grp)
                    grp["metas"].append(prev_b)
                    if len(grp["metas"]) == G or prev_a is None:
                        emit_ln(grp)
                        pending.extend(grp["metas"])
                        grp = None
                prev_b = prev_a
                prev_a = meta

            while pending:
                emit_mv(pending.popleft())

            # ---- writeback ----
            osb = outp.tile([2, K], F32, name="osb")
            for h in (0, 1):
                nc.vector.tensor_copy(osb[:, h * KH:(h + 1) * KH],
                                      pmv[h][:, :])
            nc.sync.dma_start(out=out_d[:, :], in_=osb[:, :])
            _stack.close()

    nc.compile()
    return nc


_NC_CACHE = None


def _get_nc():
    global _NC_CACHE
    if _NC_CACHE is None:
        _NC_CACHE = build_nc()
    return _NC_CACHE


def make_in_maps(desc0, desc1, logic0, logic1,
                 W1_0, b1_0, W2_0, b2_0, W3_0, b3_0,
                 W1_1, b1_1, W2_1, b2_1, W3_1, b3_1):
    bf16 = ml_dtypes.bfloat16
    fp8 = ml_dtypes.float8_e4m3
    NPAD = NCORES * NA

    per_species = {}
    for s, (desc, logic, W1, b1v, W2, b2v, W3, b3v) in enumerate((
            (desc0, logic0, W1_0, b1_0, W2_0, b2_0, W3_0, b3_0),
            (desc1, logic1, W1_1, b1_1, W2_1, b2_1, W3_1, b3_1))):
        descT = np.zeros((D, NPAD), dtype=bf16)
        descT[:, :N] = np.asarray(desc, np.float32).T.astype(bf16)
        logicT = np.zeros((NPAD, K), dtype=fp8)
        logicT[:N, :] = np.asarray(logic, np.float32).T.astype(fp8)
        per_species[s] = dict(
            descT=descT,
            logicT=logicT,
            w1t=np.ascontiguousarray(np.asarray(W1, np.float32).T).astype(bf16),
            w2t=np.ascontiguousarray(
                np.asarray(W2, np.float32).T.reshape(2, 128, H)
                .transpose(1, 0, 2)).astype(bf16),
            w3t=np.ascontiguousarray(
                np.asarray(W3, np.float32).T.reshape(2, 128, 2)
                .transpose(1, 0, 2)).astype(bf16),
            b1=np.ascontiguousarray(
                np.asarray(b1v, np.float32).reshape(2, 128).T),
            b2=np.ascontiguousarray(
                np.asarray(b2v, np.float32).reshape(2, 128).T),
            b3=np.ascontiguousarray(
                np.broadcast_to(np.asarray(b3v, np.float32), (128, 2))),
        )

    in_maps = []
    for c in range(NCORES):
        m = {}
        for s in (0, 1):
            sp = per_species[s]
            m[f"descT{s}"] = sp["descT"][:, c * NA:(c + 1) * NA]
            m[f"logicT{s}"] = sp["logicT"][c * NA:(c + 1) * NA, :]
            m[f"w1t{s}"] = sp["w1t"]
            m[f"w2t{s}"] = sp["w2t"]
            m[f"w3t{s}"] = sp["w3t"]
            m[f"b1{s}"] = sp["b1"]
            m[f"b2{s}"] = sp["b2"]
            m[f"b3{s}"] = sp["b3"]
        in_maps.append(m)
    return in_maps


def run(in_maps, trace=False, **kwargs):
    nc = _get_nc()
    return run_bass_kernel_spmd(nc, in_maps, core_ids=list(range(NCORES)),
                                trace=trace, **kwargs)


def kernel(**inputs):
    in_maps = make_in_maps(**inputs)
    res = run(in_maps)
    total = np.zeros((2, K), np.float64)
    for r in res.results:
        total += r["out"].astype(np.float64)
    return np.concatenate([total[0], total[1]]).astype(np.float32)
